# revision 13
# baseline (speedup 1.0000x reference)
"""ARD kernel matrix on 8 TRN2 NeuronCores.

k(x, y) = exp(-0.5 * sum_d (x_d - y_d)^2 / bw_d),  bw = exp(log_bw)

Sharding: 4x2 grid over the [4096, 4096] output. Core c = (mg, ng) computes
the [1024, 2048] tile for x rows [mg*1024, +1024) and y rows [ng*2048, +2048).

Per-core device program (all engines overlapped via Tile):
  - s = exp(-0.5 * lbw) on ACT, per-partition scale of the D-major (transposed)
    x/y operands on DVE.
  - squared row norms via ones-vector matmuls on PE (reduction over the
    partition/contraction dim).
  - cross = xw^T yw via bf16 matmuls, fp32 PSUM accumulation; an augmented
    K=1 matmul adds -0.5*y2[j] into the same PSUM tile.
  - one ACT pass: out = exp(psum + bias) with per-partition bias -0.5*x2[i],
    written directly as bf16 and DMA'd out.
"""

import sys

import numpy as np

if "/opt/trn_rl_repo" not in sys.path:
    sys.path.insert(0, "/opt/trn_rl_repo")

import ml_dtypes

N, M, D = 4096, 4096, 256
MG, NG = 4, 2  # core grid: MG x-row groups x NG y-row groups
NL, ML = N // MG, M // NG  # per-core output tile: [1024, 2048]
KC = D // 128  # contraction chunks of 128
N_CORES = 8

_CACHE = {}
LAST_RESULT = None  # BassKernelResults of the most recent run (for profiling)


def _ensure_profile_hook():
    """Register the axon NTFF profile hook if the image's antenv lacks it.

    Only affects runs with BASS_TRACE=1; without it run_bass_kernel_spmd
    never consults the hook. Failures degrade to no-profile silently.
    """
    try:
        import contextlib
        import ctypes
        import types

        try:
            from antenv.axon_hooks import get_axon_ntff_profile_hook  # noqa: F401

            return  # real module present
        except ImportError:
            pass

        so_path = "/opt/axon/libaxon_pjrt.so"
        lib = ctypes.CDLL(so_path)
        if not hasattr(lib, "axon_start_nrt_profile"):
            return
        lib.axon_start_nrt_profile.argtypes = [
            ctypes.POINTER(ctypes.c_int64),
            ctypes.c_size_t,
        ]
        lib.axon_start_nrt_profile.restype = ctypes.c_int64
        lib.axon_stop_nrt_profile.argtypes = [ctypes.c_char_p]
        lib.axon_stop_nrt_profile.restype = ctypes.c_int64

        @contextlib.contextmanager
        def _hook(output_dir, device_ids):
            import jax

            jax.devices()
            if device_ids:
                ids = (ctypes.c_int64 * len(device_ids))(*device_ids)
                rc = lib.axon_start_nrt_profile(ids, len(device_ids))
            else:
                rc = lib.axon_start_nrt_profile(None, 0)
            if rc != 0:
                raise RuntimeError(f"axon_start_nrt_profile rc={rc}")
            try:
                yield
            finally:
                n = lib.axon_stop_nrt_profile(str(output_dir).encode())
                print(f"profile: {n} file(s) written to {output_dir}", file=sys.stderr)

        mod = types.ModuleType("antenv.axon_hooks")
        mod.get_axon_ntff_profile_hook = lambda: _hook
        mod.set_axon_ntff_profile_hook = lambda h: None
        sys.modules["antenv.axon_hooks"] = mod

        # artifact upload needs bucket creds this container may not have
        from concourse import bass_utils as _bu

        _bu.upload_artifacts = lambda tmpdir: tmpdir
    except Exception as e:  # pragma: no cover - profiling is best-effort
        print(f"profile hook setup failed: {e}", file=sys.stderr)


def _build_nc():
    from contextlib import ExitStack

    import concourse.tile as tile
    from concourse import bacc, mybir

    dt = mybir.dt
    FP32 = dt.float32
    BF16 = dt.bfloat16
    Act = mybir.ActivationFunctionType

    nc = bacc.Bacc()
    xt_d = nc.declare_dram_parameter("xt", [D, NL], BF16, isOutput=False)
    yt_d = nc.declare_dram_parameter("yt", [D, ML], BF16, isOutput=False)
    lbw_d = nc.declare_dram_parameter("lbw", [128, KC], FP32, isOutput=False)
    out_d = nc.declare_dram_parameter("out", [NL, ML], BF16, isOutput=True)

    n_mt = NL // 128  # 8 output row tiles
    n_ns = ML // 512  # 4 psum-bank columns per output row tile

    NSW = 512  # matmul moving free-dim (one PSUM bank)
    n_sw = ML // NSW

    with tile.TileContext(nc) as tc, ExitStack() as ctx:
        cpool = ctx.enter_context(tc.tile_pool(name="const", bufs=1))
        work = ctx.enter_context(tc.tile_pool(name="work", bufs=2))
        outp = ctx.enter_context(tc.tile_pool(name="outp", bufs=4))
        psum = ctx.enter_context(tc.tile_pool(name="psum", bufs=2, space="PSUM"))

        # out[i,j] = exp(cross_w[i,j] - 0.5*x2[i] - 0.5*y2[j]) in ONE ACT pass:
        #   cross_w = (s2 x)^T y accumulated in PSUM (bf16 matmuls)
        #   -0.5*y2[j] added into PSUM by an augmented K=1 matmul
        #   -0.5*x2[i] applied as the ACT per-partition bias

        # --- s2 = exp(-lbw), [128, KC] ---
        lbw_sb = cpool.tile([128, KC], FP32)
        nc.sync.dma_start(lbw_sb[:], lbw_d[:])
        s2_f = cpool.tile([128, KC], FP32)
        nc.scalar.activation(s2_f[:], lbw_sb[:], Act.Exp, scale=-1.0)
        s2_b = cpool.tile([128, KC], BF16)
        nc.vector.tensor_copy(s2_b[:], s2_f[:])

        # --- x side first: its chain gates the first ACT exp ---
        xw2 = []
        xsq = []
        for k in range(KC):
            xr = cpool.tile([128, NL], BF16, tag=f"xraw{k}")
            nc.sync.dma_start(xr[:], xt_d[128 * k : 128 * (k + 1), :])
            xsq_k = work.tile([128, NL], BF16, tag=f"xsq{k}")
            nc.vector.tensor_mul(xsq_k[:], xr[:], xr[:])
            xw2_k = cpool.tile([128, NL], BF16, tag=f"xw2{k}")
            nc.vector.tensor_scalar_mul(xw2_k[:], xr[:], s2_f[:, k : k + 1])
            xw2.append(xw2_k)
            xsq.append(xsq_k)

        yraw = []
        ysq = []
        for k in range(KC):
            yr = cpool.tile([128, ML], BF16, tag=f"yraw{k}")
            nc.sync.dma_start(yr[:], yt_d[128 * k : 128 * (k + 1), :])
            ysq_k = work.tile([128, ML], BF16, tag=f"ysq{k}")
            for h in range(2):
                sl = slice(1024 * h, 1024 * (h + 1))
                nc.vector.tensor_mul(ysq_k[:, sl], yr[:, sl], yr[:, sl])
            yraw.append(yr)
            ysq.append(ysq_k)

        ones_row = cpool.tile([1, 128], BF16)
        nc.vector.memset(ones_row[:], 1.0)

        # --- x2 -> negx2 bias (DVE copy out of PSUM) ---
        pp = psum.tile([128, ML], mybir.dt.float32, tag="ps")
        for m in range(n_mt):
            for k in range(KC):
                nc.tensor.matmul(
                    pp[:, m : m + 1],
                    lhsT=xsq[k][:, 128 * m : 128 * (m + 1)],
                    rhs=s2_b[:, k : k + 1],
                    start=(k == 0),
                    stop=(k == KC - 1),
                )
        negx2 = cpool.tile([128, n_mt], FP32)
        nc.vector.tensor_scalar_mul(negx2[:], pp[:, 0:n_mt], -0.5)

        # --- main m=0 cross matmuls (keeps PE busy while y2 prep lands) ---
        def main_mms(ps, m):
            for ns in range(n_sw):
                for k in range(KC):
                    nc.tensor.matmul(
                        ps[:, NSW * ns : NSW * (ns + 1)],
                        lhsT=xw2[k][:, 128 * m : 128 * (m + 1)],
                        rhs=yraw[k][:, NSW * ns : NSW * (ns + 1)],
                        start=(k == 0),
                        stop=False,
                    )

        def aug_mms(ps, negy2):
            for ns in range(n_sw):
                nc.tensor.matmul(
                    ps[:, NSW * ns : NSW * (ns + 1)],
                    lhsT=ones_row[:],
                    rhs=negy2[0:1, NSW * ns : NSW * (ns + 1)],
                    start=False,
                    stop=True,
                )

        ps0 = psum.tile([128, ML], mybir.dt.float32, tag="ps")
        main_mms(ps0, 0)

        # --- y2 -> negy2 row (DVE scaled copy out of PSUM) ---
        pp2 = psum.tile([128, ML], mybir.dt.float32, tag="ps")
        for ns in range(n_ns):
            for k in range(KC):
                nc.tensor.matmul(
                    pp2[0:1, 512 * ns : 512 * (ns + 1)],
                    lhsT=s2_b[:, k : k + 1],
                    rhs=ysq[k][:, 512 * ns : 512 * (ns + 1)],
                    start=(k == 0),
                    stop=(k == KC - 1),
                )
        negy2 = cpool.tile([1, ML], BF16)
        nc.vector.tensor_scalar_mul(negy2[:], pp2[0:1, :], -0.5)

        # --- finish m=0, then the rest ---
        aug_mms(ps0, negy2)
        ob = outp.tile([128, ML], BF16, tag="ob")
        nc.scalar.activation(ob[:], ps0[:], Act.Exp, bias=negx2[:, 0:1], scale=1.0)
        nc.sync.dma_start(out_d[0:128, :], ob[:])

        for m in range(1, n_mt):
            ps = psum.tile([128, ML], mybir.dt.float32, tag="ps")
            main_mms(ps, m)
            aug_mms(ps, negy2)
            ob = outp.tile([128, ML], BF16, tag="ob")
            nc.scalar.activation(
                ob[:], ps[:], Act.Exp, bias=negx2[:, m : m + 1], scale=1.0
            )
            nc.sync.dma_start(out_d[128 * m : 128 * (m + 1), :], ob[:])

    nc.finalize()
    return nc


def _get_nc():
    if "nc" not in _CACHE:
        _CACHE["nc"] = _build_nc()
    return _CACHE["nc"]


def kernel(x, y, log_band_width):
    global LAST_RESULT
    _ensure_profile_hook()
    from concourse.bass_utils import run_bass_kernel_spmd

    nc = _get_nc()

    xtb = np.ascontiguousarray(x.astype(ml_dtypes.bfloat16).T)  # [D, N]
    ytb = np.ascontiguousarray(y.astype(ml_dtypes.bfloat16).T)  # [D, M]
    # lbw_t[p, k] = lbw[128k + p] so column k scales contraction chunk k
    lbw_t = np.ascontiguousarray(
        log_band_width.astype(np.float32).reshape(KC, 128).T
    )

    in_maps = []
    for c in range(N_CORES):
        mg, ng = divmod(c, NG)
        in_maps.append(
            {
                "xt": np.ascontiguousarray(xtb[:, mg * NL : (mg + 1) * NL]),
                "yt": np.ascontiguousarray(ytb[:, ng * ML : (ng + 1) * ML]),
                "lbw": lbw_t,
            }
        )

    res = run_bass_kernel_spmd(nc, in_maps, core_ids=list(range(N_CORES)))
    LAST_RESULT = res

    outs = [np.asarray(res.results[c]["out"]) for c in range(N_CORES)]
    rows = [
        np.concatenate([outs[mg * NG + ng] for ng in range(NG)], axis=1)
        for mg in range(MG)
    ]
    return np.concatenate(rows, axis=0).astype(np.float32)
